# revision 7
# baseline (speedup 1.0000x reference)
"""Causal self-attention (QK-RMSNorm + rotary, H=16, D=1024, B=2, T=2048) on 8 NeuronCores.

Sharding: core c handles batch b = c // 4 and heads 4*(c%4) .. 4*(c%4)+3.
Each core computes the qkv projection for its heads, causal attention, and a
row-parallel slice of the output projection; the host sums the 4 partial
outputs per batch element.

Device layout is feature-major ([dim, token]); the host pre-transposes x and
the weight slices so no on-chip transposes of x are needed. Rotary is applied
via a second projection against sign-permuted weight columns (rot(q) = x @
W_rot), so all vector ops stay partition-aligned. Softmax needs no running
max: RMS-normalized q,k bound scores to |s| <= sqrt(d_head) * ||q|| = 8.
"""
import sys
sys.path.insert(0, '/opt/trn_rl_repo')

import numpy as np
from contextlib import ExitStack

import concourse.bass as bass
import concourse.tile as tile
from concourse import bacc, mybir
from concourse.bass_utils import run_bass_kernel_spmd

F32 = mybir.dt.float32
F32R = mybir.dt.float32r
AF = mybir.ActivationFunctionType

N_HEAD = 16
D_MODEL = 1024
D_HEAD = 64
B, T = 2, 2048
N_CORES = 8
HL = 4            # heads per core
KT = D_MODEL // 128   # 8 contraction tiles
NCH = T // 512    # 4 t-chunks per core
NIB = T // 512    # 4 i-blocks
NTT = T // 128    # 16 t-tiles
SCALE = D_HEAD ** -0.5

_cached = {}


def _build():
    nc = bacc.Bacc("TRN2", target_bir_lowering=False, debug=False,
                   num_devices=N_CORES)

    # ---- DRAM I/O ----------------------------------------------------------
    xT = nc.dram_tensor("xT", [D_MODEL, T], F32R, kind="ExternalInput").ap()
    wqk = nc.dram_tensor("wqk", [D_MODEL, 512], F32R, kind="ExternalInput").ap()
    wqkr = nc.dram_tensor("wqkr", [D_MODEL, 512], F32R, kind="ExternalInput").ap()
    wv = nc.dram_tensor("wv", [D_MODEL, 256], F32R, kind="ExternalInput").ap()
    wp = nc.dram_tensor("wp", [HL, 64, 1024], F32R, kind="ExternalInput").ap()
    cosT = nc.dram_tensor("cosT", [128, T], F32, kind="ExternalInput").ap()
    sinT = nc.dram_tensor("sinT", [128, T], F32, kind="ExternalInput").ap()
    trimask = nc.dram_tensor("trimask", [128, 128], F32R, kind="ExternalInput").ap()
    rsel = nc.dram_tensor("rsel", [128, 2], F32R, kind="ExternalInput").ap()
    rselT = nc.dram_tensor("rselT", [2, 128], F32R, kind="ExternalInput").ap()
    ones64 = nc.dram_tensor("ones64", [1, 64], F32R, kind="ExternalInput").ap()
    onescol = nc.dram_tensor("onescol", [128, HL * NTT], F32R,
                             kind="ExternalInput").ap()
    ident = nc.dram_tensor("ident", [128, 128], F32R, kind="ExternalInput").ap()
    out = nc.dram_tensor("out", [T, D_MODEL], F32, kind="ExternalOutput").ap()

    with tile.TileContext(nc) as tc, ExitStack() as ctx:
        ctx.enter_context(nc.allow_low_precision(
            reason="float32r tiles share fp32 bit layout; matmul runs fp32r"))

        # ---- long-lived pools ---------------------------------------------
        cpool = ctx.enter_context(tc.tile_pool(name="consts", bufs=1))
        ppool = ctx.enter_context(tc.tile_pool(name="persist", bufs=1))
        epool = ctx.enter_context(tc.tile_pool(name="exps", bufs=3))
        ps_a = ctx.enter_context(tc.tile_pool(name="psa", bufs=2, space="PSUM"))
        ps_b = ctx.enter_context(tc.tile_pool(name="psb", bufs=2, space="PSUM"))
        ps_c = ctx.enter_context(tc.tile_pool(name="psc", bufs=2, space="PSUM"))
        ps_y = ctx.enter_context(tc.tile_pool(name="psy", bufs=2, space="PSUM"))

        tri_sb = cpool.tile([128, 128], F32R)
        nc.sync.dma_start(tri_sb[:], trimask[:])
        rsel_sb = cpool.tile([128, 2], F32R)
        nc.sync.dma_start(rsel_sb[:], rsel[:])
        rselT_sb = cpool.tile([2, 128], F32R)
        nc.sync.dma_start(rselT_sb[:], rselT[:])
        ones64_sb = cpool.tile([1, 64], F32R)
        nc.sync.dma_start(ones64_sb[:], ones64[:])
        id_sb = cpool.tile([128, 128], F32R)
        nc.sync.dma_start(id_sb[:], ident[:])

        # persistent activations
        qT_sb = [ppool.tile([128, T], F32R, tag=f"qT{t}", name=f"qT{t}")
                 for t in range(2)]
        kT_sb = [ppool.tile([128, T], F32R, tag=f"kT{t}", name=f"kT{t}")
                 for t in range(2)]
        v_sb = ppool.tile([128, HL * NTT * 65], F32R, tag="v")
        v_blk = v_sb[:].rearrange("p (g o) -> p g o", o=65)
        nc.sync.dma_start(v_blk[:, :, 64:65], onescol.unsqueeze(2))
        yT_sb = [ppool.tile([64, T], F32R, tag=f"yT{h}", name=f"yT{h}")
                 for h in range(HL)]

        # ---- phase 1+2: projections + rmsnorm + rope + v transpose ---------
        with tc.tile_pool(name="wts", bufs=1) as wtp, \
             tc.tile_pool(name="xtp", bufs=2) as xpool, \
             tc.tile_pool(name="pwork", bufs=2) as wpool:
            wqk_sb = wtp.tile([128, KT, 512], F32R)
            nc.sync.dma_start(wqk_sb[:], wqk.rearrange("(k p) m -> p k m", p=128))
            wqkr_sb = wtp.tile([128, KT, 512], F32R)
            nc.sync.dma_start(wqkr_sb[:], wqkr.rearrange("(k p) m -> p k m", p=128))
            wv_sb = wtp.tile([128, KT, 256], F32R)
            nc.sync.dma_start(wv_sb[:], wv.rearrange("(k p) m -> p k m", p=128))
            cos_sb = wtp.tile([128, T], F32)
            nc.sync.dma_start(cos_sb[:], cosT[:])
            sin_sb = wtp.tile([128, T], F32)
            nc.sync.dma_start(sin_sb[:], sinT[:])

            for ch in range(NCH):
                cs = ch * 512
                xt = xpool.tile([128, KT, 512], F32R, tag="xt")
                nc.sync.dma_start(xt[:], xT[:, cs:cs + 512]
                                  .rearrange("(k p) n -> p k n", p=128))

                # q/k M-tiles: 0,1 = q head-pairs; 2,3 = k head-pairs
                for mt in range(4):
                    dst = qT_sb[mt] if mt < 2 else kT_sb[mt - 2]
                    acc = ps_a.tile([128, 512], F32, tag="qk")
                    accr = ps_b.tile([128, 512], F32, tag="qkr")
                    for k in range(KT):
                        nc.tensor.matmul(acc[:],
                                         wqk_sb[:, k, mt * 128:(mt + 1) * 128],
                                         xt[:, k, :], start=(k == 0),
                                         stop=(k == KT - 1))
                    for k in range(KT):
                        nc.tensor.matmul(accr[:],
                                         wqkr_sb[:, k, mt * 128:(mt + 1) * 128],
                                         xt[:, k, :], start=(k == 0),
                                         stop=(k == KT - 1))
                    t1 = wpool.tile([128, 512], F32, tag="t1")
                    nc.vector.tensor_mul(t1[:], acc[:], cos_sb[:, cs:cs + 512])
                    t2 = wpool.tile([128, 512], F32, tag="t2")
                    nc.vector.tensor_mul(t2[:], accr[:], sin_sb[:, cs:cs + 512])
                    qr = wpool.tile([128, 512], F32, tag="qr")
                    nc.vector.tensor_add(qr[:], t1[:], t2[:])
                    # sum of squares per 64-row half (rope preserves norms);
                    # rsqrt = exp(-0.5 ln) keeps all ACT funcs in one table set
                    sq = wpool.tile([128, 512], F32R, tag="sq")
                    nc.scalar.square(sq[:], qr[:])
                    ssq = ps_c.tile([2, 512], F32, tag="stat")
                    nc.tensor.matmul(ssq[:], rsel_sb[:], sq[:], start=True,
                                     stop=True)
                    lnv = wpool.tile([2, 512], F32, tag="lnv")
                    nc.scalar.activation(lnv[:], ssq[:], AF.Ln, scale=1.0 / 64.0)
                    rms = wpool.tile([2, 512], F32R, tag="rms")
                    nc.scalar.activation(rms[:], lnv[:], AF.Exp, scale=-0.5)
                    bc = ps_c.tile([128, 512], F32, tag="stat")
                    nc.tensor.matmul(bc[:], rselT_sb[:], rms[:], start=True,
                                     stop=True)
                    nc.vector.tensor_mul(dst[:, cs:cs + 512], qr[:], bc[:])

                # v: project feature-major, then transpose to token-major
                for mt in range(2):
                    accv = ps_a.tile([128, 512], F32, tag="qk")
                    for k in range(KT):
                        nc.tensor.matmul(accv[:],
                                         wv_sb[:, k, mt * 128:(mt + 1) * 128],
                                         xt[:, k, :], start=(k == 0),
                                         stop=(k == KT - 1))
                    vtc = wpool.tile([128, 512], F32R, tag="vtc")
                    nc.scalar.copy(vtc[:], accv[:])
                    tps = ps_b.tile([128, 512], F32R, tag="qkr")
                    for s in range(4):
                        nc.tensor.transpose(tps[:, s * 128:(s + 1) * 128],
                                            vtc[:, s * 128:(s + 1) * 128],
                                            id_sb[:])
                    o = tps[:].rearrange("p (s h d) -> p s h d", s=4, h=2)
                    for h2 in range(2):
                        h = mt * 2 + h2
                        dst = v_blk[:, h * NTT + ch * 4:h * NTT + ch * 4 + 4,
                                    0:64]
                        nc.scalar.copy(dst, o[:, :, h2, :])

        # ---- phase 3+4: attention, then output projection ------------------
        with tc.tile_pool(name="wpp", bufs=1) as wpp, \
             tc.tile_pool(name="awork", bufs=2) as awork:
            wp_sb = [wpp.tile([64, 1024], F32R, tag=f"wp{h}", name=f"wp{h}")
                     for h in range(HL)]
            for h in range(HL):
                nc.sync.dma_start(wp_sb[h][:], wp[h])

            for h in range(HL):
                hh, ht = h % 2, h // 2
                qs = qT_sb[ht][hh * 64:(hh + 1) * 64, :]
                ks = kT_sb[ht][hh * 64:(hh + 1) * 64, :]
                for ib in range(NIB):
                    ibs = ib * 512
                    njt = 4 * (ib + 1)
                    yacc = ps_y.tile([65, 512], F32, tag="yacc")
                    for jt in range(njt):
                        o = max(0, jt * 128 - ibs)
                        w = 512 - o
                        s_ps = ps_b.tile([128, 512], F32, tag="qkr")
                        nc.tensor.matmul(s_ps[:, 0:w],
                                         ks[:, jt * 128:(jt + 1) * 128],
                                         qs[:, ibs + o:ibs + 512],
                                         start=True, stop=True)
                        p_sb = epool.tile([128, 512], F32R, tag="p")
                        nc.scalar.activation(p_sb[:, 0:w], s_ps[:, 0:w], AF.Exp,
                                             scale=SCALE)
                        if jt * 128 >= ibs:  # diagonal tile: triangular mask
                            nc.vector.tensor_mul(p_sb[:, 0:128], p_sb[:, 0:128],
                                                 tri_sb[:])
                        nc.tensor.matmul(yacc[0:65, o:512],
                                         v_blk[:, h * NTT + jt, :],
                                         p_sb[:, 0:w],
                                         start=(jt == 0), stop=(jt == njt - 1))
                    # softmax division, deferred to the y write
                    rec = awork.tile([1, 512], F32R, tag="rec")
                    nc.vector.reciprocal(rec[:], yacc[64:65, :])
                    bc2 = ps_c.tile([64, 512], F32, tag="stat")
                    nc.tensor.matmul(bc2[:], ones64_sb[:], rec[:], start=True,
                                     stop=True)
                    bc2s = awork.tile([64, 512], F32, tag="bc2s")
                    nc.scalar.copy(bc2s[:], bc2[:])
                    nc.vector.tensor_mul(yT_sb[h][:, ibs:ibs + 512],
                                         yacc[0:64, :], bc2s[:])

            # output projection (row-parallel over this core's head dims)
            for mt in range(NTT):
                for oc in range(2):
                    acc = ps_a.tile([128, 512], F32, tag="qk")
                    for h in range(HL):
                        nc.tensor.matmul(acc[:],
                                         yT_sb[h][:, mt * 128:(mt + 1) * 128],
                                         wp_sb[h][:, oc * 512:(oc + 1) * 512],
                                         start=(h == 0), stop=(h == HL - 1))
                    o_sb = awork.tile([128, 512], F32, tag="osb")
                    nc.vector.tensor_copy(o_sb[:], acc[:])
                    nc.sync.dma_start(out[mt * 128:(mt + 1) * 128,
                                          oc * 512:(oc + 1) * 512], o_sb[:])

    nc.compile()
    return nc


def _host_inputs(x, w_attn, w_proj):
    """Build the 8 per-core input maps."""
    inv_freq = 1.0 / (10000.0 ** (np.arange(0, D_HEAD, 2, dtype=np.float32)
                                  / D_HEAD))
    t = np.arange(T, dtype=np.float32)
    freqs = np.einsum('i,j->ij', t, inv_freq)          # [T, 32]
    cos64 = np.cos(np.concatenate([freqs, freqs], 1)).T  # [64, T]
    sin64 = np.sin(np.concatenate([freqs, freqs], 1)).T
    cosT = np.concatenate([cos64, cos64], 0).astype(np.float32)  # [128, T]
    sinT = np.concatenate([sin64, sin64], 0).astype(np.float32)

    tri = (np.arange(128)[:, None] <= np.arange(128)[None, :]).astype(np.float32)
    rsel = np.zeros((128, 2), np.float32)
    rsel[:64, 0] = 1.0
    rsel[64:, 1] = 1.0
    ident = np.eye(128, dtype=np.float32)
    ones64 = np.ones((1, 64), np.float32)
    onescol = np.ones((128, HL * NTT), np.float32)

    wq = w_attn[:D_MODEL]          # [1024, 1024] rows: head h = 64h..64h+63
    wk = w_attn[D_MODEL:2 * D_MODEL]
    wv_full = w_attn[2 * D_MODEL:]

    def rot_rows(w):
        # rows of w are per-head output dims; rot(q)[d] = -q[d+32] / q[d-32]
        w = w.reshape(N_HEAD, D_HEAD, D_MODEL)
        wr = np.concatenate([-w[:, 32:, :], w[:, :32, :]], axis=1)
        return wr.reshape(N_HEAD * D_HEAD, D_MODEL)

    wqr_full = rot_rows(wq)
    wkr_full = rot_rows(wk)

    in_maps = []
    for c in range(N_CORES):
        b, hg = c // 4, c % 4
        hs = slice(hg * 4 * D_HEAD, (hg * 4 + 4) * D_HEAD)   # 256 rows
        wqk_c = np.concatenate([wq[hs], wk[hs]], 0).T.copy()       # [1024, 512]
        wqkr_c = np.concatenate([wqr_full[hs], wkr_full[hs]], 0).T.copy()
        wv_c = wv_full[hs].T.copy()                                # [1024, 256]
        wp_c = np.stack([w_proj[:, (hg * 4 + j) * D_HEAD:
                                (hg * 4 + j + 1) * D_HEAD].T
                         for j in range(HL)])                      # [4, 64, 1024]
        in_maps.append({
            "xT": np.ascontiguousarray(x[b].T),
            "wqk": np.ascontiguousarray(wqk_c),
            "wqkr": np.ascontiguousarray(wqkr_c),
            "wv": np.ascontiguousarray(wv_c),
            "wp": np.ascontiguousarray(wp_c),
            "cosT": cosT, "sinT": sinT, "trimask": tri,
            "rsel": rsel, "rselT": np.ascontiguousarray(rsel.T),
            "ones64": ones64, "onescol": onescol, "ident": ident,
        })
    return in_maps


def kernel(x, w_attn, w_proj, _want_results=False):
    x = np.asarray(x, dtype=np.float32)
    w_attn = np.asarray(w_attn, dtype=np.float32)
    w_proj = np.asarray(w_proj, dtype=np.float32)

    if "nc" not in _cached:
        _cached["nc"] = _build()
    nc = _cached["nc"]

    in_maps = _host_inputs(x, w_attn, w_proj)
    res = run_bass_kernel_spmd(nc, in_maps, list(range(N_CORES)))

    full = np.zeros((B, T, D_MODEL), np.float32)
    for c in range(N_CORES):
        full[c // 4] += res.results[c]["out"]
    if _want_results:
        return full, res
    return full


# revision 9
# speedup vs baseline: 1.1878x; 1.1878x over previous
"""Causal self-attention (QK-RMSNorm + rotary, H=16, D=1024, B=2, T=2048) on 8 NeuronCores.

Sharding: core c handles batch b = c // 4 and heads 4*(c%4) .. 4*(c%4)+3.
Each core computes the qkv projection for its heads, causal attention, and a
row-parallel slice of the output projection; the host sums the 4 partial
outputs per batch element.

Device layout is feature-major ([dim, token]); the host pre-transposes x and
the weight slices so no on-chip transposes of x are needed. Rotary is applied
via a second projection against sign-permuted weight columns (rot(q) = x @
W_rot), so all vector ops stay partition-aligned. Softmax needs no running
max: RMS-normalized q,k bound scores to |s| <= sqrt(d_head) * ||q|| = 8.

All matmuls keep K=128 and N>=256 (fp32r fast path): q is stored zero-padded
per head so scores contract the full 128 partitions; v blocks are sliced 128
wide (trailing columns are don't-care rows in PSUM); y is written in head
pairs, with odd heads reading the previous v-block's ones column so their
softmax sum lands on a partition the pair layout can use.
"""
import sys
sys.path.insert(0, '/opt/trn_rl_repo')

import numpy as np
from contextlib import ExitStack

import concourse.bass as bass
import concourse.tile as tile
from concourse import bacc, mybir
from concourse.bass_utils import run_bass_kernel_spmd

F32 = mybir.dt.float32
F32R = mybir.dt.float32r
AF = mybir.ActivationFunctionType

N_HEAD = 16
D_MODEL = 1024
D_HEAD = 64
B, T = 2, 2048
N_CORES = 8
HL = 4            # heads per core
KT = D_MODEL // 128   # 8 contraction tiles
NCH = T // 512    # 4 t-chunks per core
NIB = T // 512    # 4 i-blocks
NTT = T // 128    # 16 t-tiles
SCALE = D_HEAD ** -0.5

_cached = {}


def _build():
    nc = bacc.Bacc("TRN2", target_bir_lowering=False, debug=False,
                   num_devices=N_CORES)

    # ---- DRAM I/O ----------------------------------------------------------
    xT = nc.dram_tensor("xT", [D_MODEL, T], F32R, kind="ExternalInput").ap()
    wqk = nc.dram_tensor("wqk", [D_MODEL, 512], F32R, kind="ExternalInput").ap()
    wqkr = nc.dram_tensor("wqkr", [D_MODEL, 512], F32R, kind="ExternalInput").ap()
    wv = nc.dram_tensor("wv", [D_MODEL, 256], F32R, kind="ExternalInput").ap()
    wpP = nc.dram_tensor("wpP", [2, 128, 1024], F32R, kind="ExternalInput").ap()
    cosT = nc.dram_tensor("cosT", [128, T], F32, kind="ExternalInput").ap()
    sinT = nc.dram_tensor("sinT", [128, T], F32, kind="ExternalInput").ap()
    trimask = nc.dram_tensor("trimask", [128, 128], F32R, kind="ExternalInput").ap()
    rsel = nc.dram_tensor("rsel", [128, 2], F32R, kind="ExternalInput").ap()
    rselT = nc.dram_tensor("rselT", [2, 128], F32R, kind="ExternalInput").ap()
    sel16p = nc.dram_tensor("sel16p", [16, 8 * 128], F32R,
                            kind="ExternalInput").ap()
    onescol = nc.dram_tensor("onescol", [128, HL * NTT], F32R,
                             kind="ExternalInput").ap()
    zpad = nc.dram_tensor("zpad", [64, T], F32R, kind="ExternalInput").ap()
    ident = nc.dram_tensor("ident", [128, 128], F32R, kind="ExternalInput").ap()
    out = nc.dram_tensor("out", [T, D_MODEL], F32, kind="ExternalOutput").ap()

    with tile.TileContext(nc) as tc, ExitStack() as ctx:
        ctx.enter_context(nc.allow_low_precision(
            reason="float32r tiles share fp32 bit layout; matmul runs fp32r"))

        cpool = ctx.enter_context(tc.tile_pool(name="consts", bufs=1))
        ppool = ctx.enter_context(tc.tile_pool(name="persist", bufs=1))
        epool = ctx.enter_context(tc.tile_pool(name="exps", bufs=3))
        ps_a = ctx.enter_context(tc.tile_pool(name="psa", bufs=2, space="PSUM"))
        ps_b = ctx.enter_context(tc.tile_pool(name="psb", bufs=2, space="PSUM"))
        ps_c = ctx.enter_context(tc.tile_pool(name="psc", bufs=2, space="PSUM"))
        ps_y = ctx.enter_context(tc.tile_pool(name="psy", bufs=2, space="PSUM"))

        tri_sb = cpool.tile([128, 128], F32R)
        nc.sync.dma_start(tri_sb[:], trimask[:])
        rsel_sb = cpool.tile([128, 2], F32R)
        nc.sync.dma_start(rsel_sb[:], rsel[:])
        rselT_sb = cpool.tile([2, 128], F32R)
        nc.sync.dma_start(rselT_sb[:], rselT[:])
        sel16p_sb = cpool.tile([16, 8 * 128], F32R)
        nc.sync.dma_start(sel16p_sb[:], sel16p[:])
        id_sb = cpool.tile([128, 128], F32R)
        nc.sync.dma_start(id_sb[:], ident[:])

        # persistent activations: zero-padded per-head q, paired k, v blocks
        # of [64 dims | ones] with 64 pad columns at the end, paired y
        qTz = [ppool.tile([128, T], F32R, tag=f"qTz{h}", name=f"qTz{h}")
               for h in range(HL)]
        for h in range(HL):
            half = slice(64, 128) if h % 2 == 0 else slice(0, 64)
            nc.sync.dma_start(qTz[h][half, :], zpad[:])
        kT_sb = [ppool.tile([128, T], F32R, tag=f"kT{t}", name=f"kT{t}")
                 for t in range(2)]
        v_sb = ppool.tile([128, HL * NTT * 65 + 64], F32R, tag="v")
        v_blk = v_sb[:, 0:HL * NTT * 65].rearrange("p (g o) -> p g o", o=65)
        nc.sync.dma_start(v_blk[:, :, 64:65], onescol.unsqueeze(2))
        yP = [ppool.tile([128, T], F32R, tag=f"yP{t}", name=f"yP{t}")
              for t in range(2)]

        # ---- phase 1: projections + rmsnorm + rope + v transpose -----------
        with tc.tile_pool(name="wts", bufs=1) as wtp, \
             tc.tile_pool(name="xtp", bufs=2) as xpool, \
             tc.tile_pool(name="pwork", bufs=2) as wpool:
            wqk_sb = wtp.tile([128, KT, 512], F32R)
            wqkr_sb = wtp.tile([128, KT, 512], F32R)
            wv_sb = wtp.tile([128, KT, 256], F32R)
            for k in range(KT):
                ks = slice(k * 128, (k + 1) * 128)
                nc.sync.dma_start(wqk_sb[:, k, :], wqk[ks, :])
                nc.sync.dma_start(wqkr_sb[:, k, :], wqkr[ks, :])
                nc.sync.dma_start(wv_sb[:, k, :], wv[ks, :])
            cos_sb = wtp.tile([128, T], F32)
            nc.sync.dma_start(cos_sb[:], cosT[:])
            sin_sb = wtp.tile([128, T], F32)
            nc.sync.dma_start(sin_sb[:], sinT[:])

            for ch in range(NCH):
                cs = ch * 512
                xt = xpool.tile([128, KT, 512], F32R, tag="xt")
                for k in range(KT):
                    nc.sync.dma_start(xt[:, k, :],
                                      xT[k * 128:(k + 1) * 128, cs:cs + 512])

                # q/k M-tiles: 0,1 = q head-pairs; 2,3 = k head-pairs
                for mt in range(4):
                    acc = ps_a.tile([128, 512], F32, tag="qk")
                    accr = ps_b.tile([128, 512], F32, tag="qkr")
                    for k in range(KT):
                        nc.tensor.matmul(acc[:],
                                         wqk_sb[:, k, mt * 128:(mt + 1) * 128],
                                         xt[:, k, :], start=(k == 0),
                                         stop=(k == KT - 1))
                    for k in range(KT):
                        nc.tensor.matmul(accr[:],
                                         wqkr_sb[:, k, mt * 128:(mt + 1) * 128],
                                         xt[:, k, :], start=(k == 0),
                                         stop=(k == KT - 1))
                    t1 = wpool.tile([128, 512], F32, tag="t1")
                    nc.vector.tensor_mul(t1[:], acc[:], cos_sb[:, cs:cs + 512])
                    t2 = wpool.tile([128, 512], F32, tag="t2")
                    nc.vector.tensor_mul(t2[:], accr[:], sin_sb[:, cs:cs + 512])
                    qr = t1
                    nc.vector.tensor_add(qr[:], t1[:], t2[:])
                    # sum of squares per 64-row half (rope preserves norms);
                    # rsqrt = exp(-0.5 ln) keeps ACT funcs near one table set
                    sq = wpool.tile([128, 512], F32R, tag="sq")
                    nc.scalar.square(sq[:], qr[:])
                    ssq = ps_c.tile([2, 512], F32, tag="stat")
                    nc.tensor.matmul(ssq[:], rsel_sb[:], sq[:], start=True,
                                     stop=True)
                    lnv = wpool.tile([2, 512], F32, tag="lnv")
                    nc.scalar.activation(lnv[:], ssq[:], AF.Ln, scale=1.0 / 64.0)
                    rms = wpool.tile([2, 512], F32R, tag="rms")
                    nc.scalar.activation(rms[:], lnv[:], AF.Exp, scale=-0.5)
                    bc = ps_c.tile([128, 512], F32, tag="stat")
                    nc.tensor.matmul(bc[:], rselT_sb[:], rms[:], start=True,
                                     stop=True)
                    if mt < 2:
                        # q: split the scaled write into zero-padded per-head
                        nc.vector.tensor_mul(qTz[2 * mt][0:64, cs:cs + 512],
                                             qr[0:64, :], bc[0:64, :])
                        nc.vector.tensor_mul(qTz[2 * mt + 1][64:128, cs:cs + 512],
                                             qr[64:128, :], bc[64:128, :])
                    else:
                        nc.vector.tensor_mul(kT_sb[mt - 2][:, cs:cs + 512],
                                             qr[:], bc[:])

                # v: project feature-major, then transpose to token-major
                for mt in range(2):
                    accv = ps_a.tile([128, 512], F32, tag="qk")
                    for k in range(KT):
                        nc.tensor.matmul(accv[:],
                                         wv_sb[:, k, mt * 128:(mt + 1) * 128],
                                         xt[:, k, :], start=(k == 0),
                                         stop=(k == KT - 1))
                    vtc = wpool.tile([128, 512], F32R, tag="vtc")
                    nc.scalar.copy(vtc[:], accv[:])
                    tps = ps_b.tile([128, 512], F32R, tag="qkr")
                    for s in range(4):
                        nc.tensor.transpose(tps[:, s * 128:(s + 1) * 128],
                                            vtc[:, s * 128:(s + 1) * 128],
                                            id_sb[:])
                    o = tps[:].rearrange("p (s h d) -> p s h d", s=4, h=2)
                    for h2 in range(2):
                        h = mt * 2 + h2
                        dst = v_blk[:, h * NTT + ch * 4:h * NTT + ch * 4 + 4,
                                    0:64]
                        nc.scalar.copy(dst, o[:, :, h2, :])

        # ---- phase 2: attention, batched softmax division, out proj --------
        with tc.tile_pool(name="wpp", bufs=1) as wpp, \
             tc.tile_pool(name="ysg", bufs=1) as ysgp, \
             tc.tile_pool(name="awork", bufs=2) as awork:
            wpP_sb = [wpp.tile([128, 1024], F32R, tag=f"wpP{t}", name=f"wpP{t}")
                      for t in range(2)]
            for t in range(2):
                nc.sync.dma_start(wpP_sb[t][:], wpP[t])
            sums_all = wpp.tile([16, 512], F32, tag="sums")
            ySG = [ysgp.tile([128, 512], F32, tag=f"ySG{r}", name=f"ySG{r}")
                   for r in range(16)]

            for h in range(HL):
                ht, hh = h // 2, h % 2
                for ib in range(NIB):
                    r = h * NIB + ib
                    ibs = ib * 512
                    njt = 4 * (ib + 1)
                    yacc = ps_y.tile([128, 512], F32, tag="yacc")
                    for jt in range(njt):
                        o = max(0, jt * 128 - ibs)
                        w = 512 - o
                        s_ps = ps_b.tile([128, 512], F32, tag="qkr")
                        nc.tensor.matmul(s_ps[:, 0:w],
                                         kT_sb[ht][:, jt * 128:(jt + 1) * 128],
                                         qTz[h][:, ibs + o:ibs + 512],
                                         start=True, stop=True)
                        p_sb = epool.tile([128, 512], F32R, tag="p")
                        nc.scalar.activation(p_sb[:, 0:w], s_ps[:, 0:w], AF.Exp,
                                             scale=SCALE)
                        if jt * 128 >= ibs:  # diagonal tile: triangular mask
                            nc.vector.tensor_mul(p_sb[:, 0:128], p_sb[:, 0:128],
                                                 tri_sb[:])
                        g = h * NTT + jt
                        if hh == 0:
                            vau = v_sb[:, g * 65:g * 65 + 128]  # y@0-63, sum@64
                        else:
                            vau = v_sb[:, g * 65 - 64:g * 65 + 64]  # sum@63, y@64+
                        nc.tensor.matmul(yacc[:, o:512], vau, p_sb[:, 0:w],
                                         start=(jt == 0), stop=(jt == njt - 1))
                    # stage y + its softmax sums to SBUF; collect sums by DMA
                    nc.scalar.copy(ySG[r][:], yacc[:])
                    srow = 64 if hh == 0 else 63
                    nc.sync.dma_start(sums_all[r:r + 1, :],
                                      ySG[r][srow:srow + 1, :])

            # batched softmax division into paired y
            recip_all = awork.tile([16, 512], F32R, tag="recip")
            nc.vector.reciprocal(recip_all[:], sums_all[:])
            for hp in range(2):
                for ib in range(NIB):
                    bc2 = ps_c.tile([128, 512], F32, tag="stat")
                    nc.tensor.matmul(
                        bc2[:], sel16p_sb[:, (hp * NIB + ib) * 128:
                                          (hp * NIB + ib + 1) * 128],
                        recip_all[:], start=True, stop=True)
                    re, ro = 2 * hp * NIB + ib, (2 * hp + 1) * NIB + ib
                    ibs = ib * 512
                    nc.vector.tensor_mul(yP[hp][0:64, ibs:ibs + 512],
                                         ySG[re][0:64, :], bc2[0:64, :])
                    nc.vector.tensor_mul(yP[hp][64:128, ibs:ibs + 512],
                                         ySG[ro][64:128, :], bc2[64:128, :])

            # output projection (row-parallel over this core's head dims)
            for mt in range(NTT):
                for oc in range(2):
                    acc = ps_a.tile([128, 512], F32, tag="qk")
                    for t in range(2):
                        nc.tensor.matmul(acc[:],
                                         yP[t][:, mt * 128:(mt + 1) * 128],
                                         wpP_sb[t][:, oc * 512:(oc + 1) * 512],
                                         start=(t == 0), stop=(t == 1))
                    o_sb = awork.tile([128, 512], F32, tag="osb")
                    nc.vector.tensor_copy(o_sb[:], acc[:])
                    nc.sync.dma_start(out[mt * 128:(mt + 1) * 128,
                                          oc * 512:(oc + 1) * 512], o_sb[:])

    nc.compile()
    return nc


def _host_inputs(x, w_attn, w_proj):
    """Build the 8 per-core input maps."""
    inv_freq = 1.0 / (10000.0 ** (np.arange(0, D_HEAD, 2, dtype=np.float32)
                                  / D_HEAD))
    t = np.arange(T, dtype=np.float32)
    freqs = np.einsum('i,j->ij', t, inv_freq)          # [T, 32]
    cos64 = np.cos(np.concatenate([freqs, freqs], 1)).T  # [64, T]
    sin64 = np.sin(np.concatenate([freqs, freqs], 1)).T
    cosT = np.concatenate([cos64, cos64], 0).astype(np.float32)  # [128, T]
    sinT = np.concatenate([sin64, sin64], 0).astype(np.float32)

    tri = (np.arange(128)[:, None] <= np.arange(128)[None, :]).astype(np.float32)
    rsel = np.zeros((128, 2), np.float32)
    rsel[:64, 0] = 1.0
    rsel[64:, 1] = 1.0
    # sel16p[(hp,ib) block]: rows 0-63 pick sums row of even head, 64-127 odd
    sel16p = np.zeros((16, 8 * 128), np.float32)
    for hp in range(2):
        for ib in range(NIB):
            blk = (hp * NIB + ib) * 128
            sel16p[(2 * hp) * NIB + ib, blk:blk + 64] = 1.0
            sel16p[(2 * hp + 1) * NIB + ib, blk + 64:blk + 128] = 1.0
    ident = np.eye(128, dtype=np.float32)
    onescol = np.ones((128, HL * NTT), np.float32)
    zpad = np.zeros((64, T), np.float32)

    wq = w_attn[:D_MODEL]          # [1024, 1024] rows: head h = 64h..64h+63
    wk = w_attn[D_MODEL:2 * D_MODEL]
    wv_full = w_attn[2 * D_MODEL:]

    def rot_rows(w):
        # rows of w are per-head output dims; rot(q)[d] = -q[d+32] / q[d-32]
        w = w.reshape(N_HEAD, D_HEAD, D_MODEL)
        wr = np.concatenate([-w[:, 32:, :], w[:, :32, :]], axis=1)
        return wr.reshape(N_HEAD * D_HEAD, D_MODEL)

    wqr_full = rot_rows(wq)
    wkr_full = rot_rows(wk)

    in_maps = []
    for c in range(N_CORES):
        b, hg = c // 4, c % 4
        hs = slice(hg * 4 * D_HEAD, (hg * 4 + 4) * D_HEAD)   # 256 rows
        wqk_c = np.concatenate([wq[hs], wk[hs]], 0).T.copy()       # [1024, 512]
        wqkr_c = np.concatenate([wqr_full[hs], wkr_full[hs]], 0).T.copy()
        wv_c = wv_full[hs].T.copy()                                # [1024, 256]
        wp_c = [w_proj[:, (hg * 4 + j) * D_HEAD:(hg * 4 + j + 1) * D_HEAD].T
                for j in range(HL)]                                # 4x[64,1024]
        wpP_c = np.stack([np.concatenate([wp_c[0], wp_c[1]], 0),
                          np.concatenate([wp_c[2], wp_c[3]], 0)])  # [2,128,1024]
        in_maps.append({
            "xT": np.ascontiguousarray(x[b].T),
            "wqk": np.ascontiguousarray(wqk_c),
            "wqkr": np.ascontiguousarray(wqkr_c),
            "wv": np.ascontiguousarray(wv_c),
            "wpP": np.ascontiguousarray(wpP_c),
            "cosT": cosT, "sinT": sinT, "trimask": tri,
            "rsel": rsel, "rselT": np.ascontiguousarray(rsel.T),
            "sel16p": sel16p, "onescol": onescol, "zpad": zpad,
            "ident": ident,
        })
    return in_maps


def kernel(x, w_attn, w_proj, _want_results=False):
    x = np.asarray(x, dtype=np.float32)
    w_attn = np.asarray(w_attn, dtype=np.float32)
    w_proj = np.asarray(w_proj, dtype=np.float32)

    if "nc" not in _cached:
        _cached["nc"] = _build()
    nc = _cached["nc"]

    in_maps = _host_inputs(x, w_attn, w_proj)
    res = run_bass_kernel_spmd(nc, in_maps, list(range(N_CORES)))

    full = np.zeros((B, T, D_MODEL), np.float32)
    for c in range(N_CORES):
        full[c // 4] += res.results[c]["out"]
    if _want_results:
        return full, res
    return full


# revision 11
# speedup vs baseline: 1.3468x; 1.1338x over previous
"""Causal self-attention (QK-RMSNorm + rotary, H=16, D=1024, B=2, T=2048) on 8 NeuronCores.

Sharding: core c handles batch b = c // 4 and heads 4*(c%4) .. 4*(c%4)+3.
Each core computes the qkv projection for its heads, causal attention, and a
row-parallel slice of the output projection; the host sums the 4 partial
outputs per batch element.

Device layout is feature-major ([dim, token]); the host pre-transposes x and
the weight slices so no on-chip transposes of x are needed. Rotary is applied
via a second projection against sign-permuted weight columns (rot(q) = x @
W_rot), so all vector ops stay partition-aligned. Softmax needs no running
max: RMS-normalized q,k bound scores to |s| <= sqrt(d_head) * ||q|| = 8.

All matmuls keep K=128 and N>=256 (fp32r fast path): q is stored zero-padded
per head so scores contract the full 128 partitions; v blocks are sliced 128
wide (trailing columns are don't-care rows in PSUM); y is written in head
pairs, with odd heads reading the previous v-block's ones column so their
softmax sum lands on a partition the pair layout can use.
"""
import sys
sys.path.insert(0, '/opt/trn_rl_repo')

import numpy as np
from contextlib import ExitStack

import concourse.bass as bass
import concourse.tile as tile
from concourse import bacc, mybir
from concourse.bass_utils import run_bass_kernel_spmd

F32 = mybir.dt.float32
F32R = mybir.dt.float32r
AF = mybir.ActivationFunctionType

N_HEAD = 16
D_MODEL = 1024
D_HEAD = 64
B, T = 2, 2048
N_CORES = 8
HL = 4            # heads per core
KT = D_MODEL // 128   # 8 contraction tiles
NCH = T // 512    # 4 t-chunks per core
NIB = T // 512    # 4 i-blocks
NTT = T // 128    # 16 t-tiles
SCALE = D_HEAD ** -0.5

_cached = {}


def _build():
    nc = bacc.Bacc("TRN2", target_bir_lowering=False, debug=False,
                   num_devices=N_CORES)

    # ---- DRAM I/O ----------------------------------------------------------
    xT = nc.dram_tensor("xT", [D_MODEL, T], F32R, kind="ExternalInput").ap()
    wqk = nc.dram_tensor("wqk", [D_MODEL, 512], F32R, kind="ExternalInput").ap()
    wqkr = nc.dram_tensor("wqkr", [D_MODEL, 512], F32R, kind="ExternalInput").ap()
    wv = nc.dram_tensor("wv", [D_MODEL, 256], F32R, kind="ExternalInput").ap()
    wpP = nc.dram_tensor("wpP", [2, 128, 1024], F32R, kind="ExternalInput").ap()
    cosT = nc.dram_tensor("cosT", [128, T], F32, kind="ExternalInput").ap()
    sinT = nc.dram_tensor("sinT", [128, T], F32, kind="ExternalInput").ap()
    trimask = nc.dram_tensor("trimask", [128, 128], F32R, kind="ExternalInput").ap()
    rsel32 = nc.dram_tensor("rsel32", [128, 16 * 32], F32R,
                            kind="ExternalInput").ap()
    rselT32 = nc.dram_tensor("rselT32", [32, 16 * 128], F32R,
                             kind="ExternalInput").ap()
    sel16p = nc.dram_tensor("sel16p", [16, 8 * 128], F32R,
                            kind="ExternalInput").ap()
    onescol = nc.dram_tensor("onescol", [128, HL * NTT], F32R,
                             kind="ExternalInput").ap()
    zpad = nc.dram_tensor("zpad", [64, T], F32R, kind="ExternalInput").ap()
    ident = nc.dram_tensor("ident", [128, 128], F32R, kind="ExternalInput").ap()
    out = nc.dram_tensor("out", [T, D_MODEL], F32, kind="ExternalOutput").ap()

    with tile.TileContext(nc) as tc, ExitStack() as ctx:
        ctx.enter_context(nc.allow_low_precision(
            reason="float32r tiles share fp32 bit layout; matmul runs fp32r"))

        cpool = ctx.enter_context(tc.tile_pool(name="consts", bufs=1))
        ppool = ctx.enter_context(tc.tile_pool(name="persist", bufs=1))
        epool = ctx.enter_context(tc.tile_pool(name="exps", bufs=3))
        ps_a = ctx.enter_context(tc.tile_pool(name="psa", bufs=2, space="PSUM"))
        ps_b = ctx.enter_context(tc.tile_pool(name="psb", bufs=2, space="PSUM"))
        ps_c = ctx.enter_context(tc.tile_pool(name="psc", bufs=2, space="PSUM"))
        ps_y = ctx.enter_context(tc.tile_pool(name="psy", bufs=2, space="PSUM"))

        tri_sb = cpool.tile([128, 128], F32R)
        rsel32_sb = cpool.tile([128, 16 * 32], F32R)
        nc.sync.dma_start(rsel32_sb[:], rsel32[:])
        rselT32_sb = cpool.tile([32, 16 * 128], F32R)
        nc.sync.dma_start(rselT32_sb[:], rselT32[:])
        sel16p_sb = cpool.tile([16, 8 * 128], F32R)
        id_sb = cpool.tile([128, 128], F32R)
        nc.sync.dma_start(id_sb[:], ident[:])

        # persistent activations: zero-padded per-head q, paired k, v blocks
        # of [64 dims | ones] with 64 pad columns at the end, paired y
        qTz = [ppool.tile([128, T], F32R, tag=f"qTz{h}", name=f"qTz{h}")
               for h in range(HL)]
        kT_sb = [ppool.tile([128, T], F32R, tag=f"kT{t}", name=f"kT{t}")
                 for t in range(2)]
        v_sb = ppool.tile([128, HL * NTT * 65 + 64], F32R, tag="v")
        v_blk = v_sb[:, 0:HL * NTT * 65].rearrange("p (g o) -> p g o", o=65)
        yP = [ppool.tile([128, T], F32R, tag=f"yP{t}", name=f"yP{t}")
              for t in range(2)]

        # ---- phase 1: projections + rmsnorm + rope + v transpose -----------
        with tc.tile_pool(name="wts", bufs=1) as wtp, \
             tc.tile_pool(name="xtp", bufs=2) as xpool, \
             tc.tile_pool(name="pwork", bufs=2) as wpool:
            wqk_sb = wtp.tile([128, KT, 512], F32R)
            wqkr_sb = wtp.tile([128, KT, 512], F32R)
            wv_sb = wtp.tile([128, KT, 256], F32R)
            for k in range(KT):
                ks = slice(k * 128, (k + 1) * 128)
                nc.sync.dma_start(wqk_sb[:, k, :], wqk[ks, :])
                nc.sync.dma_start(wqkr_sb[:, k, :], wqkr[ks, :])
                nc.sync.dma_start(wv_sb[:, k, :], wv[ks, :])
            cos_sb = wtp.tile([128, T], F32)
            nc.sync.dma_start(cos_sb[:], cosT[:])
            sin_sb = wtp.tile([128, T], F32)
            nc.sync.dma_start(sin_sb[:], sinT[:])
            ssq_all = ps_c.tile([32, 512], F32, tag="stat", name="ssq_all")

            for ch in range(NCH):
                cs = ch * 512
                xt = xpool.tile([128, KT, 512], F32R, tag="xt")
                for k in range(KT):
                    nc.sync.dma_start(xt[:, k, :],
                                      xT[k * 128:(k + 1) * 128, cs:cs + 512])

                # q/k M-tiles: 0,1 = q head-pairs; 2,3 = k head-pairs
                for mt in range(4):
                    acc = ps_a.tile([128, 512], F32, tag="qk")
                    accr = ps_b.tile([128, 512], F32, tag="qkr")
                    for k in range(KT):
                        nc.tensor.matmul(acc[:],
                                         wqk_sb[:, k, mt * 128:(mt + 1) * 128],
                                         xt[:, k, :], start=(k == 0),
                                         stop=(k == KT - 1))
                    for k in range(KT):
                        nc.tensor.matmul(accr[:],
                                         wqkr_sb[:, k, mt * 128:(mt + 1) * 128],
                                         xt[:, k, :], start=(k == 0),
                                         stop=(k == KT - 1))
                    t1 = wpool.tile([128, 512], F32, tag="t1")
                    nc.vector.tensor_mul(t1[:], acc[:], cos_sb[:, cs:cs + 512])
                    t2 = wpool.tile([128, 512], F32, tag="t2")
                    nc.vector.tensor_mul(t2[:], accr[:], sin_sb[:, cs:cs + 512])
                    # rope output written unscaled; rms scale applied in-place
                    # after the batched ln/exp pass (one ACT table set swap)
                    if mt < 2:
                        dsts = [qTz[2 * mt][0:64, cs:cs + 512],
                                qTz[2 * mt + 1][64:128, cs:cs + 512]]
                        nc.vector.tensor_add(dsts[0], t1[0:64, :], t2[0:64, :])
                        nc.vector.tensor_add(dsts[1], t1[64:128, :],
                                             t2[64:128, :])
                    else:
                        dsts = [kT_sb[mt - 2][:, cs:cs + 512]]
                        nc.vector.tensor_add(dsts[0], t1[:], t2[:])
                    sq = wpool.tile([128, 512], F32R, tag="sq")
                    for d in dsts:
                        b0 = d.base_partition() if callable(d.base_partition) \
                            else d.base_partition
                        nc.scalar.square(sq[b0:b0 + d.shape[0], :], d)
                    idx = ch * 4 + mt
                    nc.tensor.matmul(ssq_all[:],
                                     rsel32_sb[:, idx * 32:(idx + 1) * 32],
                                     sq[:], start=(idx == 0), stop=(idx == 15))

                # v: project feature-major, then transpose to token-major
                for mt in range(2):
                    accv = ps_a.tile([128, 512], F32, tag="qk")
                    for k in range(KT):
                        nc.tensor.matmul(accv[:],
                                         wv_sb[:, k, mt * 128:(mt + 1) * 128],
                                         xt[:, k, :], start=(k == 0),
                                         stop=(k == KT - 1))
                    vtc = wpool.tile([128, 512], F32R, tag="vtc")
                    nc.scalar.copy(vtc[:], accv[:])
                    tps = ps_b.tile([128, 512], F32R, tag="qkr")
                    for s in range(4):
                        nc.tensor.transpose(tps[:, s * 128:(s + 1) * 128],
                                            vtc[:, s * 128:(s + 1) * 128],
                                            id_sb[:])
                    o = tps[:].rearrange("p (s h d) -> p s h d", s=4, h=2)
                    for h2 in range(2):
                        h = mt * 2 + h2
                        dst = v_blk[:, h * NTT + ch * 4:h * NTT + ch * 4 + 4,
                                    0:64]
                        nc.scalar.copy(dst, o[:, :, h2, :])

            # batched rsqrt = exp(-0.5 ln(ms)) over all 32 (tile, half) rows
            lnv_all = wpool.tile([32, 512], F32, tag="t1")
            nc.scalar.activation(lnv_all[:], ssq_all[:], AF.Ln, scale=1.0 / 64.0)
            rms_all = wpool.tile([32, 512], F32R, tag="t2")
            nc.scalar.activation(rms_all[:], lnv_all[:], AF.Exp, scale=-0.5)
            for ch in range(NCH):
                cs = ch * 512
                for mt in range(4):
                    idx = ch * 4 + mt
                    bc = ps_c.tile([128, 512], F32, tag="stat")
                    nc.tensor.matmul(bc[:],
                                     rselT32_sb[:, idx * 128:(idx + 1) * 128],
                                     rms_all[:], start=True, stop=True)
                    if mt < 2:
                        dsts = [qTz[2 * mt][0:64, cs:cs + 512],
                                qTz[2 * mt + 1][64:128, cs:cs + 512]]
                    else:
                        dsts = [kT_sb[mt - 2][:, cs:cs + 512]]
                    for d in dsts:
                        b0 = d.base_partition() if callable(d.base_partition) \
                            else d.base_partition
                        nc.vector.tensor_mul(d, d, bc[b0:b0 + d.shape[0], :])

        # deferred constant loads (not needed until attention)
        for h in range(HL):
            half = slice(64, 128) if h % 2 == 0 else slice(0, 64)
            nc.sync.dma_start(qTz[h][half, :], zpad[:])
        nc.sync.dma_start(v_blk[:, :, 64:65], onescol.unsqueeze(2))
        nc.sync.dma_start(tri_sb[:], trimask[:])
        nc.sync.dma_start(sel16p_sb[:], sel16p[:])

        # ---- phase 2: attention, batched softmax division, out proj --------
        with tc.tile_pool(name="wpp", bufs=1) as wpp, \
             tc.tile_pool(name="ysg", bufs=1) as ysgp, \
             tc.tile_pool(name="awork", bufs=2) as awork:
            wpP_sb = [wpp.tile([128, 1024], F32R, tag=f"wpP{t}", name=f"wpP{t}")
                      for t in range(2)]
            for t in range(2):
                nc.sync.dma_start(wpP_sb[t][:], wpP[t])
            sums_all = wpp.tile([16, 512], F32, tag="sums")
            ySG = [ysgp.tile([128, 512], F32, tag=f"ySG{r}", name=f"ySG{r}")
                   for r in range(16)]

            for h in range(HL):
                ht, hh = h // 2, h % 2
                for ib in range(NIB):
                    r = h * NIB + ib
                    ibs = ib * 512
                    njt = 4 * (ib + 1)
                    yacc = ps_y.tile([128, 512], F32, tag="yacc")
                    for jt in range(njt):
                        o = max(0, jt * 128 - ibs)
                        w = 512 - o
                        s_ps = ps_b.tile([128, 512], F32, tag="qkr")
                        nc.tensor.matmul(s_ps[:, 0:w],
                                         kT_sb[ht][:, jt * 128:(jt + 1) * 128],
                                         qTz[h][:, ibs + o:ibs + 512],
                                         start=True, stop=True)
                        p_sb = epool.tile([128, 512], F32R, tag="p")
                        nc.scalar.activation(p_sb[:, 0:w], s_ps[:, 0:w], AF.Exp,
                                             scale=SCALE)
                        if jt * 128 >= ibs:  # diagonal tile: triangular mask
                            nc.vector.tensor_mul(p_sb[:, 0:128], p_sb[:, 0:128],
                                                 tri_sb[:])
                        g = h * NTT + jt
                        if hh == 0:
                            vau = v_sb[:, g * 65:g * 65 + 128]  # y@0-63, sum@64
                        else:
                            vau = v_sb[:, g * 65 - 64:g * 65 + 64]  # sum@63, y@64+
                        nc.tensor.matmul(yacc[:, o:512], vau, p_sb[:, 0:w],
                                         start=(jt == 0), stop=(jt == njt - 1))
                    # stage y + its softmax sums to SBUF; collect sums by DMA
                    nc.scalar.copy(ySG[r][:], yacc[:])
                    srow = 64 if hh == 0 else 63
                    nc.sync.dma_start(sums_all[r:r + 1, :],
                                      ySG[r][srow:srow + 1, :])

            # batched softmax division into paired y
            recip_all = awork.tile([16, 512], F32R, tag="recip")
            nc.vector.reciprocal(recip_all[:], sums_all[:])
            for hp in range(2):
                for ib in range(NIB):
                    bc2 = ps_c.tile([128, 512], F32, tag="stat")
                    nc.tensor.matmul(
                        bc2[:], sel16p_sb[:, (hp * NIB + ib) * 128:
                                          (hp * NIB + ib + 1) * 128],
                        recip_all[:], start=True, stop=True)
                    re, ro = 2 * hp * NIB + ib, (2 * hp + 1) * NIB + ib
                    ibs = ib * 512
                    nc.vector.tensor_mul(yP[hp][0:64, ibs:ibs + 512],
                                         ySG[re][0:64, :], bc2[0:64, :])
                    nc.vector.tensor_mul(yP[hp][64:128, ibs:ibs + 512],
                                         ySG[ro][64:128, :], bc2[64:128, :])

            # output projection (row-parallel over this core's head dims)
            for mt in range(NTT):
                for oc in range(2):
                    acc = ps_a.tile([128, 512], F32, tag="qk")
                    for t in range(2):
                        nc.tensor.matmul(acc[:],
                                         yP[t][:, mt * 128:(mt + 1) * 128],
                                         wpP_sb[t][:, oc * 512:(oc + 1) * 512],
                                         start=(t == 0), stop=(t == 1))
                    o_sb = awork.tile([128, 512], F32, tag="osb")
                    nc.vector.tensor_copy(o_sb[:], acc[:])
                    nc.sync.dma_start(out[mt * 128:(mt + 1) * 128,
                                          oc * 512:(oc + 1) * 512], o_sb[:])

    nc.compile()
    return nc


def _host_inputs(x, w_attn, w_proj):
    """Build the 8 per-core input maps."""
    inv_freq = 1.0 / (10000.0 ** (np.arange(0, D_HEAD, 2, dtype=np.float32)
                                  / D_HEAD))
    t = np.arange(T, dtype=np.float32)
    freqs = np.einsum('i,j->ij', t, inv_freq)          # [T, 32]
    cos64 = np.cos(np.concatenate([freqs, freqs], 1)).T  # [64, T]
    sin64 = np.sin(np.concatenate([freqs, freqs], 1)).T
    cosT = np.concatenate([cos64, cos64], 0).astype(np.float32)  # [128, T]
    sinT = np.concatenate([sin64, sin64], 0).astype(np.float32)

    tri = (np.arange(128)[:, None] <= np.arange(128)[None, :]).astype(np.float32)
    rsel32 = np.zeros((128, 16 * 32), np.float32)
    rselT32 = np.zeros((32, 16 * 128), np.float32)
    for chm in range(16):
        ch, mt = chm // 4, chm % 4
        for half in range(2):
            r = ch * 8 + mt * 2 + half
            ps = slice(half * 64, half * 64 + 64)
            rsel32[ps, chm * 32 + r] = 1.0
            rselT32[r, chm * 128 + half * 64:chm * 128 + half * 64 + 64] = 1.0
    # sel16p[(hp,ib) block]: rows 0-63 pick sums row of even head, 64-127 odd
    sel16p = np.zeros((16, 8 * 128), np.float32)
    for hp in range(2):
        for ib in range(NIB):
            blk = (hp * NIB + ib) * 128
            sel16p[(2 * hp) * NIB + ib, blk:blk + 64] = 1.0
            sel16p[(2 * hp + 1) * NIB + ib, blk + 64:blk + 128] = 1.0
    ident = np.eye(128, dtype=np.float32)
    onescol = np.ones((128, HL * NTT), np.float32)
    zpad = np.zeros((64, T), np.float32)

    wq = w_attn[:D_MODEL]          # [1024, 1024] rows: head h = 64h..64h+63
    wk = w_attn[D_MODEL:2 * D_MODEL]
    wv_full = w_attn[2 * D_MODEL:]

    def rot_rows(w):
        # rows of w are per-head output dims; rot(q)[d] = -q[d+32] / q[d-32]
        w = w.reshape(N_HEAD, D_HEAD, D_MODEL)
        wr = np.concatenate([-w[:, 32:, :], w[:, :32, :]], axis=1)
        return wr.reshape(N_HEAD * D_HEAD, D_MODEL)

    wqr_full = rot_rows(wq)
    wkr_full = rot_rows(wk)

    in_maps = []
    for c in range(N_CORES):
        b, hg = c // 4, c % 4
        hs = slice(hg * 4 * D_HEAD, (hg * 4 + 4) * D_HEAD)   # 256 rows
        wqk_c = np.concatenate([wq[hs], wk[hs]], 0).T.copy()       # [1024, 512]
        wqkr_c = np.concatenate([wqr_full[hs], wkr_full[hs]], 0).T.copy()
        wv_c = wv_full[hs].T.copy()                                # [1024, 256]
        wp_c = [w_proj[:, (hg * 4 + j) * D_HEAD:(hg * 4 + j + 1) * D_HEAD].T
                for j in range(HL)]                                # 4x[64,1024]
        wpP_c = np.stack([np.concatenate([wp_c[0], wp_c[1]], 0),
                          np.concatenate([wp_c[2], wp_c[3]], 0)])  # [2,128,1024]
        in_maps.append({
            "xT": np.ascontiguousarray(x[b].T),
            "wqk": np.ascontiguousarray(wqk_c),
            "wqkr": np.ascontiguousarray(wqkr_c),
            "wv": np.ascontiguousarray(wv_c),
            "wpP": np.ascontiguousarray(wpP_c),
            "cosT": cosT, "sinT": sinT, "trimask": tri,
            "rsel32": rsel32, "rselT32": rselT32,
            "sel16p": sel16p, "onescol": onescol, "zpad": zpad,
            "ident": ident,
        })
    return in_maps


def kernel(x, w_attn, w_proj, _want_results=False):
    x = np.asarray(x, dtype=np.float32)
    w_attn = np.asarray(w_attn, dtype=np.float32)
    w_proj = np.asarray(w_proj, dtype=np.float32)

    if "nc" not in _cached:
        _cached["nc"] = _build()
    nc = _cached["nc"]

    in_maps = _host_inputs(x, w_attn, w_proj)
    res = run_bass_kernel_spmd(nc, in_maps, list(range(N_CORES)))

    full = np.zeros((B, T, D_MODEL), np.float32)
    for c in range(N_CORES):
        full[c // 4] += res.results[c]["out"]
    if _want_results:
        return full, res
    return full


# revision 12
# speedup vs baseline: 1.3700x; 1.0172x over previous
"""Causal self-attention (QK-RMSNorm + rotary, H=16, D=1024, B=2, T=2048) on 8 NeuronCores.

Sharding: core c handles batch b = c // 4 and heads 4*(c%4) .. 4*(c%4)+3.
Each core computes the qkv projection for its heads, causal attention, and a
row-parallel slice of the output projection; the host sums the 4 partial
outputs per batch element.

Device layout is feature-major ([dim, token]); the host pre-transposes x and
the weight slices so no on-chip transposes of x are needed. Rotary is applied
via a second projection against sign-permuted weight columns (rot(q) = x @
W_rot), so all vector ops stay partition-aligned. Softmax needs no running
max: RMS-normalized q,k bound scores to |s| <= sqrt(d_head) * ||q|| = 8.

All matmuls keep K=128 and N>=256 (fp32r fast path): q is stored zero-padded
per head so scores contract the full 128 partitions; v blocks are sliced 128
wide (trailing columns are don't-care rows in PSUM); y is written in head
pairs, with odd heads reading the previous v-block's ones column so their
softmax sum lands on a partition the pair layout can use.
"""
import sys
sys.path.insert(0, '/opt/trn_rl_repo')

import numpy as np
from contextlib import ExitStack

import concourse.bass as bass
import concourse.tile as tile
from concourse import bacc, mybir
from concourse.bass_utils import run_bass_kernel_spmd

F32 = mybir.dt.float32
F32R = mybir.dt.float32r
AF = mybir.ActivationFunctionType

N_HEAD = 16
D_MODEL = 1024
D_HEAD = 64
B, T = 2, 2048
N_CORES = 8
HL = 4            # heads per core
KT = D_MODEL // 128   # 8 contraction tiles
NCH = T // 512    # 4 t-chunks per core
NIB = T // 512    # 4 i-blocks
NTT = T // 128    # 16 t-tiles
SCALE = D_HEAD ** -0.5

_cached = {}


def _build():
    nc = bacc.Bacc("TRN2", target_bir_lowering=False, debug=False,
                   num_devices=N_CORES)

    # ---- DRAM I/O ----------------------------------------------------------
    xT = nc.dram_tensor("xT", [D_MODEL, T], F32R, kind="ExternalInput").ap()
    wqk = nc.dram_tensor("wqk", [D_MODEL, 512], F32R, kind="ExternalInput").ap()
    wqkr = nc.dram_tensor("wqkr", [D_MODEL, 512], F32R, kind="ExternalInput").ap()
    wv = nc.dram_tensor("wv", [D_MODEL, 256], F32R, kind="ExternalInput").ap()
    wpP = nc.dram_tensor("wpP", [2, 128, 1024], F32R, kind="ExternalInput").ap()
    cosT = nc.dram_tensor("cosT", [128, T], F32, kind="ExternalInput").ap()
    sinT = nc.dram_tensor("sinT", [128, T], F32, kind="ExternalInput").ap()
    trimask = nc.dram_tensor("trimask", [128, 128], F32R, kind="ExternalInput").ap()
    rsel32 = nc.dram_tensor("rsel32", [128, 16 * 32], F32R,
                            kind="ExternalInput").ap()
    rselT32 = nc.dram_tensor("rselT32", [32, 16 * 128], F32R,
                             kind="ExternalInput").ap()
    sel16p = nc.dram_tensor("sel16p", [16, 8 * 128], F32R,
                            kind="ExternalInput").ap()
    onescol = nc.dram_tensor("onescol", [128, HL * NTT], F32R,
                             kind="ExternalInput").ap()
    zpad = nc.dram_tensor("zpad", [64, T], F32R, kind="ExternalInput").ap()
    ident = nc.dram_tensor("ident", [128, 128], F32R, kind="ExternalInput").ap()
    out = nc.dram_tensor("out", [T, D_MODEL], F32, kind="ExternalOutput").ap()

    with tile.TileContext(nc) as tc, ExitStack() as ctx:
        ctx.enter_context(nc.allow_low_precision(
            reason="float32r tiles share fp32 bit layout; matmul runs fp32r"))

        cpool = ctx.enter_context(tc.tile_pool(name="consts", bufs=1))
        ppool = ctx.enter_context(tc.tile_pool(name="persist", bufs=1))
        epool = ctx.enter_context(tc.tile_pool(name="exps", bufs=3))
        ps_a = ctx.enter_context(tc.tile_pool(name="psa", bufs=2, space="PSUM"))
        ps_b = ctx.enter_context(tc.tile_pool(name="psb", bufs=2, space="PSUM"))
        ps_c = ctx.enter_context(tc.tile_pool(name="psc", bufs=2, space="PSUM"))
        ps_y = ctx.enter_context(tc.tile_pool(name="psy", bufs=2, space="PSUM"))

        tri_sb = cpool.tile([128, 128], F32R)
        rsel32_sb = cpool.tile([128, 16 * 32], F32R)
        nc.sync.dma_start(rsel32_sb[:], rsel32[:])
        rselT32_sb = cpool.tile([32, 16 * 128], F32R)
        nc.sync.dma_start(rselT32_sb[:], rselT32[:])
        sel16p_sb = cpool.tile([16, 8 * 128], F32R)
        id_sb = cpool.tile([128, 128], F32R)
        nc.sync.dma_start(id_sb[:], ident[:])

        # persistent activations: zero-padded per-head q, paired k, v blocks
        # of [64 dims | ones] with 64 pad columns at the end, paired y
        qTz = [ppool.tile([128, T], F32R, tag=f"qTz{h}", name=f"qTz{h}")
               for h in range(HL)]
        kT_sb = [ppool.tile([128, T], F32R, tag=f"kT{t}", name=f"kT{t}")
                 for t in range(2)]
        v_sb = ppool.tile([128, HL * NTT * 65 + 64], F32R, tag="v")
        v_blk = v_sb[:, 0:HL * NTT * 65].rearrange("p (g o) -> p g o", o=65)
        yP = [ppool.tile([128, T], F32R, tag=f"yP{t}", name=f"yP{t}")
              for t in range(2)]

        # ---- phase 1: projections + rmsnorm + rope + v transpose -----------
        with tc.tile_pool(name="wts", bufs=1) as wtp, \
             tc.tile_pool(name="xtp", bufs=2) as xpool, \
             tc.tile_pool(name="pwork", bufs=2) as wpool:
            wqk_sb = wtp.tile([128, KT, 512], F32R)
            wqkr_sb = wtp.tile([128, KT, 512], F32R)
            wv_sb = wtp.tile([128, KT, 256], F32R)
            xt0 = None
            for k in range(KT):
                ks = slice(k * 128, (k + 1) * 128)
                if k == 0:
                    xt0 = xpool.tile([128, KT, 512], F32R, tag="xt", name="xt0")
                nc.sync.dma_start(xt0[:, k, :], xT[ks, 0:512])
                nc.sync.dma_start(wqk_sb[:, k, :], wqk[ks, :])
                nc.sync.dma_start(wqkr_sb[:, k, :], wqkr[ks, :])
                nc.sync.dma_start(wv_sb[:, k, :], wv[ks, :])
            cos_sb = wtp.tile([128, T], F32)
            nc.sync.dma_start(cos_sb[:], cosT[:])
            sin_sb = wtp.tile([128, T], F32)
            nc.sync.dma_start(sin_sb[:], sinT[:])
            ssq_all = ps_c.tile([32, 512], F32, tag="stat", name="ssq_all")

            for ch in range(NCH):
                cs = ch * 512
                if ch == 0:
                    xt = xt0
                else:
                    xt = xpool.tile([128, KT, 512], F32R, tag="xt")
                    for k in range(KT):
                        nc.sync.dma_start(xt[:, k, :],
                                          xT[k * 128:(k + 1) * 128, cs:cs + 512])

                # q/k M-tiles: 0,1 = q head-pairs; 2,3 = k head-pairs
                for mt in range(4):
                    acc = ps_a.tile([128, 512], F32, tag="qk")
                    accr = ps_b.tile([128, 512], F32, tag="qkr")
                    for k in range(KT):
                        nc.tensor.matmul(acc[:],
                                         wqk_sb[:, k, mt * 128:(mt + 1) * 128],
                                         xt[:, k, :], start=(k == 0),
                                         stop=(k == KT - 1))
                    for k in range(KT):
                        nc.tensor.matmul(accr[:],
                                         wqkr_sb[:, k, mt * 128:(mt + 1) * 128],
                                         xt[:, k, :], start=(k == 0),
                                         stop=(k == KT - 1))
                    t1 = wpool.tile([128, 512], F32, tag="t1")
                    nc.vector.tensor_mul(t1[:], acc[:], cos_sb[:, cs:cs + 512])
                    t2 = wpool.tile([128, 512], F32, tag="t2")
                    nc.vector.tensor_mul(t2[:], accr[:], sin_sb[:, cs:cs + 512])
                    # rope output written unscaled; rms scale applied in-place
                    # after the batched ln/exp pass (one ACT table set swap)
                    if mt < 2:
                        dsts = [qTz[2 * mt][0:64, cs:cs + 512],
                                qTz[2 * mt + 1][64:128, cs:cs + 512]]
                        nc.vector.tensor_add(dsts[0], t1[0:64, :], t2[0:64, :])
                        nc.vector.tensor_add(dsts[1], t1[64:128, :],
                                             t2[64:128, :])
                    else:
                        dsts = [kT_sb[mt - 2][:, cs:cs + 512]]
                        nc.vector.tensor_add(dsts[0], t1[:], t2[:])
                    sq = wpool.tile([128, 512], F32R, tag="sq")
                    for d in dsts:
                        b0 = d.base_partition() if callable(d.base_partition) \
                            else d.base_partition
                        nc.scalar.square(sq[b0:b0 + d.shape[0], :], d)
                    idx = ch * 4 + mt
                    nc.tensor.matmul(ssq_all[:],
                                     rsel32_sb[:, idx * 32:(idx + 1) * 32],
                                     sq[:], start=(idx == 0), stop=(idx == 15))

                # v: project feature-major, then transpose to token-major
                for mt in range(2):
                    accv = ps_a.tile([128, 512], F32, tag="qk")
                    for k in range(KT):
                        nc.tensor.matmul(accv[:],
                                         wv_sb[:, k, mt * 128:(mt + 1) * 128],
                                         xt[:, k, :], start=(k == 0),
                                         stop=(k == KT - 1))
                    vtc = wpool.tile([128, 512], F32R, tag="vtc")
                    nc.scalar.copy(vtc[:], accv[:])
                    tps = ps_b.tile([128, 512], F32R, tag="qkr")
                    for s in range(4):
                        nc.tensor.transpose(tps[:, s * 128:(s + 1) * 128],
                                            vtc[:, s * 128:(s + 1) * 128],
                                            id_sb[:])
                    o = tps[:].rearrange("p (s h d) -> p s h d", s=4, h=2)
                    for h2 in range(2):
                        h = mt * 2 + h2
                        dst = v_blk[:, h * NTT + ch * 4:h * NTT + ch * 4 + 4,
                                    0:64]
                        nc.scalar.copy(dst, o[:, :, h2, :])

            # batched rsqrt = exp(-0.5 ln(ms)) over all 32 (tile, half) rows
            lnv_all = wpool.tile([32, 512], F32, tag="t1")
            nc.scalar.activation(lnv_all[:], ssq_all[:], AF.Ln, scale=1.0 / 64.0)
            rms_all = wpool.tile([32, 512], F32R, tag="t2")
            nc.scalar.activation(rms_all[:], lnv_all[:], AF.Exp, scale=-0.5)
            for ch in range(NCH):
                cs = ch * 512
                for mt in range(4):
                    idx = ch * 4 + mt
                    bc = ps_c.tile([128, 512], F32, tag="stat")
                    nc.tensor.matmul(bc[:],
                                     rselT32_sb[:, idx * 128:(idx + 1) * 128],
                                     rms_all[:], start=True, stop=True)
                    if mt < 2:
                        dsts = [qTz[2 * mt][0:64, cs:cs + 512],
                                qTz[2 * mt + 1][64:128, cs:cs + 512]]
                    else:
                        dsts = [kT_sb[mt - 2][:, cs:cs + 512]]
                    for d in dsts:
                        b0 = d.base_partition() if callable(d.base_partition) \
                            else d.base_partition
                        nc.vector.tensor_mul(d, d, bc[b0:b0 + d.shape[0], :])

        # deferred constant loads (not needed until attention)
        for h in range(HL):
            half = slice(64, 128) if h % 2 == 0 else slice(0, 64)
            nc.sync.dma_start(qTz[h][half, :], zpad[:])
        nc.sync.dma_start(v_blk[:, :, 64:65], onescol.unsqueeze(2))
        nc.sync.dma_start(tri_sb[:], trimask[:])
        nc.sync.dma_start(sel16p_sb[:], sel16p[:])

        # ---- phase 2: attention, batched softmax division, out proj --------
        with tc.tile_pool(name="wpp", bufs=1) as wpp, \
             tc.tile_pool(name="ysg", bufs=1) as ysgp, \
             tc.tile_pool(name="awork", bufs=2) as awork:
            wpP_sb = [wpp.tile([128, 1024], F32R, tag=f"wpP{t}", name=f"wpP{t}")
                      for t in range(2)]
            for t in range(2):
                nc.sync.dma_start(wpP_sb[t][:], wpP[t])
            sums_all = wpp.tile([16, 512], F32, tag="sums")
            ySG = [ysgp.tile([128, 512], F32, tag=f"ySG{r}", name=f"ySG{r}")
                   for r in range(16)]

            for h in range(HL):
                ht, hh = h // 2, h % 2
                for ib in range(NIB):
                    r = h * NIB + ib
                    ibs = ib * 512
                    njt = 4 * (ib + 1)
                    yacc = ps_y.tile([128, 512], F32, tag="yacc")
                    for jt in range(njt):
                        o = max(0, jt * 128 - ibs)
                        w = 512 - o
                        s_ps = ps_b.tile([128, 512], F32, tag="qkr")
                        nc.tensor.matmul(s_ps[:, 0:w],
                                         kT_sb[ht][:, jt * 128:(jt + 1) * 128],
                                         qTz[h][:, ibs + o:ibs + 512],
                                         start=True, stop=True)
                        p_sb = epool.tile([128, 512], F32R, tag="p")
                        nc.scalar.activation(p_sb[:, 0:w], s_ps[:, 0:w], AF.Exp,
                                             scale=SCALE)
                        if jt * 128 >= ibs:  # diagonal tile: triangular mask
                            nc.vector.tensor_mul(p_sb[:, 0:128], p_sb[:, 0:128],
                                                 tri_sb[:])
                        g = h * NTT + jt
                        if hh == 0:
                            vau = v_sb[:, g * 65:g * 65 + 128]  # y@0-63, sum@64
                        else:
                            vau = v_sb[:, g * 65 - 64:g * 65 + 64]  # sum@63, y@64+
                        nc.tensor.matmul(yacc[:, o:512], vau, p_sb[:, 0:w],
                                         start=(jt == 0), stop=(jt == njt - 1))
                    # stage y + its softmax sums to SBUF; collect sums by DMA
                    nc.scalar.copy(ySG[r][:], yacc[:])
                    srow = 64 if hh == 0 else 63
                    nc.sync.dma_start(sums_all[r:r + 1, :],
                                      ySG[r][srow:srow + 1, :])

            # batched softmax division into paired y
            recip_all = awork.tile([16, 512], F32R, tag="recip")
            nc.vector.reciprocal(recip_all[:], sums_all[:])
            for hp in range(2):
                for ib in range(NIB):
                    bc2 = ps_c.tile([128, 512], F32, tag="stat")
                    nc.tensor.matmul(
                        bc2[:], sel16p_sb[:, (hp * NIB + ib) * 128:
                                          (hp * NIB + ib + 1) * 128],
                        recip_all[:], start=True, stop=True)
                    re, ro = 2 * hp * NIB + ib, (2 * hp + 1) * NIB + ib
                    ibs = ib * 512
                    nc.vector.tensor_mul(yP[hp][0:64, ibs:ibs + 512],
                                         ySG[re][0:64, :], bc2[0:64, :])
                    nc.vector.tensor_mul(yP[hp][64:128, ibs:ibs + 512],
                                         ySG[ro][64:128, :], bc2[64:128, :])

            # output projection (row-parallel over this core's head dims)
            for mt in range(NTT):
                for oc in range(2):
                    acc = ps_a.tile([128, 512], F32, tag="qk")
                    for t in range(2):
                        nc.tensor.matmul(acc[:],
                                         yP[t][:, mt * 128:(mt + 1) * 128],
                                         wpP_sb[t][:, oc * 512:(oc + 1) * 512],
                                         start=(t == 0), stop=(t == 1))
                    o_sb = awork.tile([128, 512], F32, tag="osb")
                    nc.vector.tensor_copy(o_sb[:], acc[:])
                    nc.sync.dma_start(out[mt * 128:(mt + 1) * 128,
                                          oc * 512:(oc + 1) * 512], o_sb[:])

    nc.compile()
    return nc


def _host_inputs(x, w_attn, w_proj):
    """Build the 8 per-core input maps."""
    inv_freq = 1.0 / (10000.0 ** (np.arange(0, D_HEAD, 2, dtype=np.float32)
                                  / D_HEAD))
    t = np.arange(T, dtype=np.float32)
    freqs = np.einsum('i,j->ij', t, inv_freq)          # [T, 32]
    cos64 = np.cos(np.concatenate([freqs, freqs], 1)).T  # [64, T]
    sin64 = np.sin(np.concatenate([freqs, freqs], 1)).T
    cosT = np.concatenate([cos64, cos64], 0).astype(np.float32)  # [128, T]
    sinT = np.concatenate([sin64, sin64], 0).astype(np.float32)

    tri = (np.arange(128)[:, None] <= np.arange(128)[None, :]).astype(np.float32)
    rsel32 = np.zeros((128, 16 * 32), np.float32)
    rselT32 = np.zeros((32, 16 * 128), np.float32)
    for chm in range(16):
        ch, mt = chm // 4, chm % 4
        for half in range(2):
            r = ch * 8 + mt * 2 + half
            ps = slice(half * 64, half * 64 + 64)
            rsel32[ps, chm * 32 + r] = 1.0
            rselT32[r, chm * 128 + half * 64:chm * 128 + half * 64 + 64] = 1.0
    # sel16p[(hp,ib) block]: rows 0-63 pick sums row of even head, 64-127 odd
    sel16p = np.zeros((16, 8 * 128), np.float32)
    for hp in range(2):
        for ib in range(NIB):
            blk = (hp * NIB + ib) * 128
            sel16p[(2 * hp) * NIB + ib, blk:blk + 64] = 1.0
            sel16p[(2 * hp + 1) * NIB + ib, blk + 64:blk + 128] = 1.0
    ident = np.eye(128, dtype=np.float32)
    onescol = np.ones((128, HL * NTT), np.float32)
    zpad = np.zeros((64, T), np.float32)

    wq = w_attn[:D_MODEL]          # [1024, 1024] rows: head h = 64h..64h+63
    wk = w_attn[D_MODEL:2 * D_MODEL]
    wv_full = w_attn[2 * D_MODEL:]

    def rot_rows(w):
        # rows of w are per-head output dims; rot(q)[d] = -q[d+32] / q[d-32]
        w = w.reshape(N_HEAD, D_HEAD, D_MODEL)
        wr = np.concatenate([-w[:, 32:, :], w[:, :32, :]], axis=1)
        return wr.reshape(N_HEAD * D_HEAD, D_MODEL)

    wqr_full = rot_rows(wq)
    wkr_full = rot_rows(wk)

    in_maps = []
    for c in range(N_CORES):
        b, hg = c // 4, c % 4
        hs = slice(hg * 4 * D_HEAD, (hg * 4 + 4) * D_HEAD)   # 256 rows
        wqk_c = np.concatenate([wq[hs], wk[hs]], 0).T.copy()       # [1024, 512]
        wqkr_c = np.concatenate([wqr_full[hs], wkr_full[hs]], 0).T.copy()
        wv_c = wv_full[hs].T.copy()                                # [1024, 256]
        wp_c = [w_proj[:, (hg * 4 + j) * D_HEAD:(hg * 4 + j + 1) * D_HEAD].T
                for j in range(HL)]                                # 4x[64,1024]
        wpP_c = np.stack([np.concatenate([wp_c[0], wp_c[1]], 0),
                          np.concatenate([wp_c[2], wp_c[3]], 0)])  # [2,128,1024]
        in_maps.append({
            "xT": np.ascontiguousarray(x[b].T),
            "wqk": np.ascontiguousarray(wqk_c),
            "wqkr": np.ascontiguousarray(wqkr_c),
            "wv": np.ascontiguousarray(wv_c),
            "wpP": np.ascontiguousarray(wpP_c),
            "cosT": cosT, "sinT": sinT, "trimask": tri,
            "rsel32": rsel32, "rselT32": rselT32,
            "sel16p": sel16p, "onescol": onescol, "zpad": zpad,
            "ident": ident,
        })
    return in_maps


def kernel(x, w_attn, w_proj, _want_results=False):
    x = np.asarray(x, dtype=np.float32)
    w_attn = np.asarray(w_attn, dtype=np.float32)
    w_proj = np.asarray(w_proj, dtype=np.float32)

    if "nc" not in _cached:
        _cached["nc"] = _build()
    nc = _cached["nc"]

    in_maps = _host_inputs(x, w_attn, w_proj)
    res = run_bass_kernel_spmd(nc, in_maps, list(range(N_CORES)))

    full = np.zeros((B, T, D_MODEL), np.float32)
    for c in range(N_CORES):
        full[c // 4] += res.results[c]["out"]
    if _want_results:
        return full, res
    return full
